# revision 34
# baseline (speedup 1.0000x reference)
"""Trainium2 Bass kernel for MQA attention (B=4, T=1024, D=2048, 16 q-heads, 1 kv-head).

Sharding: 8 cores = 4 batches x 2 head-groups (8 query heads each).
Each core computes, for its batch b and head-group g:
  - x^T is pre-transposed on the host (plain contiguous DMAs ~2x faster than
    xbar-transpose DMAs, and chunks land incrementally so the k/v/q
    projections track the DMA instead of waiting for the full tensor)
  - k/v projections (shared single KV head, duplicated across the pair),
    chunk-outer so each x chunk is consumed as it lands
  - RoPE on q/k in [H, tok] layout using host-precomputed sin/cos tables
  - causal attention in transposed-logits layout (logits^T = [k, q]) so that
    PV needs no transposes; softmax denominator rides as a fused ones-column
    of the PV rhs; no max-subtraction (logits are bounded by construction).
    Logits for a whole q-block strip go into one PSUM strip and are exp'd by
    a single ACTIVATE (the ~350-cycle per-activation overhead dominates
    small calls), with the causal mask applied post-exp only on the two
    diagonal chunks.
  - output projection for its 8 heads -> partial [T, D] in bf16, staged as
    full [128, 2048] rows so the writeback DMA uses 4KB lines
Host sums the two partials per batch (the pair all-reduce) and stacks batches.

Matmul inputs are bf16 (f32 PSUM accumulation; TensorE gets fast-weight-load
at bf16); softmax statistics and normalization stay f32.

The SPMD program is identical on all cores; only the data differs.
"""

import numpy as np
import ml_dtypes
import concourse.bass as bass
import concourse.mybir as mybir
from concourse import bacc
from concourse.tile import TileContext
from concourse.bass_utils import run_bass_kernel_spmd
from concourse.masks import make_identity
from contextlib import ExitStack

F32 = mybir.dt.float32
BF16 = mybir.dt.bfloat16
NP_BF16 = ml_dtypes.bfloat16

B, T, D, NH, HD = 4, 1024, 2048, 16, 128
HHD = HD // 2          # 64, rope half
NL = NH // 2           # 8 heads per core
DC = D // 128          # 16 contraction chunks
TT = T // 128          # 8 token tiles
EXPAD = 129            # PV rhs width: [v (128) | ones (1)]

# Rope-pair interleave: the H dim of q/k is permuted (consistently in wq/wk
# columns, host-side) so each rope pair (f, f+64) sits 16 lanes apart within
# one 32-partition quadrant; the rotate-half becomes a stream_shuffle.
SHUF_MASK = list(range(16, 32)) + list(range(16))


def _rope(nc, out, pin, cos, sin, tmp, stage):
    """RoPE in permuted [H, tok] layout. pin: [128, W] (psum f32), cos:
    duplicated cos table, sin: sign-baked sin table (-sin on first-half lanes,
    +sin on second-half lanes), tmp/stage: [128, W] f32 sbuf scratch.
    out (bf16) = pin * cos + shuffle16(pin) * sin.
    """
    nc.vector.stream_shuffle(tmp, pin, SHUF_MASK)
    nc.vector.tensor_mul(stage, pin, cos)
    nc.vector.tensor_mul(tmp, tmp, sin)
    nc.vector.tensor_add(out, stage, tmp)


def build_nc(dbg=False):
    nc = bacc.Bacc("TRN2", target_bir_lowering=False, debug=False, num_devices=8)
    dt = F32
    if dbg:
        kT_dbg = nc.dram_tensor("kT_dbg", [128, T], BF16,
                                kind="ExternalOutput").ap()
        vext_dbg = nc.dram_tensor("vext_dbg", [128, TT, EXPAD], BF16,
                                  kind="ExternalOutput").ap()
        kT_dbg2 = nc.dram_tensor("kT_dbg2", [128, T], BF16,
                                 kind="ExternalOutput").ap()
        vext_dbg2 = nc.dram_tensor("vext_dbg2", [128, TT, EXPAD], BF16,
                                   kind="ExternalOutput").ap()
        encT_dbg = nc.dram_tensor("encT_dbg", [128, NL, TT, 128], BF16,
                                  kind="ExternalOutput").ap()
        q0_dbg = nc.dram_tensor("q0_dbg", [128, T], BF16,
                                kind="ExternalOutput").ap()
        kprobe = [nc.dram_tensor(f"kprobe{i}", [128, T], BF16,
                                 kind="ExternalOutput").ap()
                  for i in range(3)]
    xT_d = nc.dram_tensor("xT", [DC, 128, T], BF16, kind="ExternalInput").ap()
    wq_d = nc.dram_tensor("wq", [NL, 128, DC, HD], BF16, kind="ExternalInput").ap()
    wk_d = nc.dram_tensor("wk", [128, DC, HD], BF16, kind="ExternalInput").ap()
    wv_d = nc.dram_tensor("wv", [128, DC, HD], BF16, kind="ExternalInput").ap()
    wo_d = nc.dram_tensor("wo", [NL, 128, D], BF16, kind="ExternalInput").ap()
    cosq_d = nc.dram_tensor("cosq", [128, T], dt, kind="ExternalInput").ap()
    sinq_d = nc.dram_tensor("sinq", [128, T], dt, kind="ExternalInput").ap()
    cosk_d = nc.dram_tensor("cosk", [128, T], dt, kind="ExternalInput").ap()
    sink_d = nc.dram_tensor("sink", [128, T], dt, kind="ExternalInput").ap()
    tri_d = nc.dram_tensor("tri", [128, 384], BF16, kind="ExternalInput").ap()
    out_d = nc.dram_tensor("out", [T, D], BF16, kind="ExternalOutput").ap()

    with TileContext(nc) as tc, ExitStack() as ctx:
        singles = ctx.enter_context(tc.tile_pool(name="singles", bufs=1))

        # one tile per D-chunk so each chunk DMA unblocks compute immediately
        xTs = [singles.tile([128, T], BF16, name=f"xT{c}") for c in range(DC)]
        kT = singles.tile([128, T], BF16)          # roped k^T
        vext = singles.tile([128, TT, EXPAD], BF16)  # v | ones column
        encT = singles.tile([128, NL, TT, 128], BF16)  # encoded^T per head, 2MB

        ident = singles.tile([128, 128], BF16)
        make_identity(nc, ident)

        # warm up the exp activation table set while DMAs land
        warm = singles.tile([128, 1], dt)
        warm2 = singles.tile([128, 1], dt)
        nc.vector.memset(warm, 0.0)
        nc.scalar.activation(out=warm2, in_=warm,
                             func=mybir.ActivationFunctionType.Exp)

        wk_sb = singles.tile([128, DC, HD], BF16)
        wv_sb = singles.tile([128, DC, HD], BF16)
        cosq = singles.tile([128, T], dt)
        sinq = singles.tile([128, T], dt)
        cosk = singles.tile([128, T], dt)
        sink = singles.tile([128, T], dt)
        tri = singles.tile([128, 384], BF16)
        wqp = ctx.enter_context(tc.tile_pool(name="wqp", bufs=NL))
        wop = ctx.enter_context(tc.tile_pool(name="wop", bufs=NL))
        wq_sbs = [wqp.tile([128, DC, HD], BF16, tag="wq", name=f"wq_t{n}")
                  for n in range(NL)]
        wo_sbs = [wop.tile([128, D], BF16, tag="wo", name=f"wo_t{n}")
                  for n in range(NL)]

        # Each HW dynamic queue (sync=q1, scalar=q10) processes its DMAs in
        # order at ~200GB/s when both pull; the gpsimd queue is
        # software-dynamic (~85GB/s) — never use it. x chunks alternate
        # between the two queues in consumption order so chunk c lands at
        # ~0.65c us and phase 1 tracks the DMA; weights/tables queue behind x
        # on each queue in needed-by order.
        nc.sync.dma_start(out=wk_sb, in_=wk_d)
        nc.scalar.dma_start(out=wv_sb, in_=wv_d)
        for c in range(0, DC, 2):
            nc.sync.dma_start(out=xTs[c], in_=xT_d[c])
            nc.scalar.dma_start(out=xTs[c + 1], in_=xT_d[c + 1])
        nc.sync.dma_start(out=cosk, in_=cosk_d)
        nc.sync.dma_start(out=sink, in_=sink_d)
        nc.scalar.dma_start(out=cosq, in_=cosq_d)
        nc.scalar.dma_start(out=sinq, in_=sinq_d)
        nc.scalar.dma_start(out=tri, in_=tri_d)
        for n in range(NL):
            nc.sync.dma_start(out=wq_sbs[n], in_=wq_d[n])
        # wo issues go on sync: DMA-issue instructions block their queue
        # when the ring fills, and the scalar queue must stay clear for the
        # phase-1 v-copies and the exps right behind them (sync has nothing
        # time-critical queued until the phase-3 output writes).
        for n in range(NL):
            nc.sync.dma_start(out=wo_sbs[n], in_=wo_d[n])

        onecol = singles.tile([128, 1], dt)
        nc.vector.memset(onecol, 1.0)

        # ---- phase 1: k^T (roped) and v_ext, chunk-outer to track the DMA ----
        with tc.tile_pool(name="pk1", bufs=1, space="PSUM") as pk1, \
             tc.tile_pool(name="pv1", bufs=1, space="PSUM") as pv1, \
             tc.tile_pool(name="ktmp", bufs=2) as ktmp:
            pk = pk1.tile([128, 1024], dt)
            pv = pv1.tile([128, 1024], dt)
            # start=True zeroes the whole 2KB psum bank ("zero region"), so
            # each bank gets exactly one start (its first matmul) and one
            # stop (its last); disjoint regions in between accumulate onto
            # the zeroed bank. pk spans 2 banks (one group each); pv packs
            # tb 0-3 in bank 0 and tb 4-7 in bank 1.
            for c in range(DC):
                st, sp = (c == 0), (c == DC - 1)
                nc.tensor.matmul(pk[:, 0:512], wk_sb[:, c, :], xTs[c][:, 0:512],
                                 start=st, stop=sp)
                nc.tensor.matmul(pk[:, 512:1024], wk_sb[:, c, :],
                                 xTs[c][:, 512:1024], start=st, stop=sp)
                for tb in range(TT):
                    nc.tensor.matmul(pv[:, tb * 128:(tb + 1) * 128],
                                     xTs[c][:, tb * 128:(tb + 1) * 128],
                                     wv_sb[:, c, :],
                                     start=(st and tb % 4 == 0),
                                     stop=(sp and tb % 4 == 3))
            for th in range(2):
                sl = slice(th * 512, (th + 1) * 512)
                tmp = ktmp.tile([128, 512], dt)
                stage = ktmp.tile([128, 512], dt, tag="stage", name="kstage")
                _rope(nc, kT[:, sl], pk[:, sl], cosk[:, sl], sink[:, sl], tmp,
                      stage)
            for tb in range(TT):
                nc.scalar.copy(out=vext[:, tb, 0:128],
                               in_=pv[:, tb * 128:(tb + 1) * 128])
                nc.scalar.copy(out=vext[:, tb, 128:129], in_=onecol)

        if dbg:
            nc.scalar.dma_start(out=kT_dbg, in_=kT)
            nc.scalar.dma_start(out=vext_dbg, in_=vext)

        # ---- phase 2: per-head q-proj + rope + causal attention ----
        with tc.tile_pool(name="qtp", bufs=2) as qtp, \
             tc.tile_pool(name="ropet", bufs=2) as ropet, \
             tc.tile_pool(name="expp", bufs=4) as expp, \
             tc.tile_pool(name="encp", bufs=3) as encp, \
             tc.tile_pool(name="recp", bufs=2) as recp, \
             tc.tile_pool(name="pq2", bufs=2, space="PSUM") as pq2, \
             tc.tile_pool(name="pl2", bufs=3, space="PSUM") as pl2, \
             tc.tile_pool(name="pe2", bufs=1, space="PSUM") as pe2, \
             tc.tile_pool(name="pt2", bufs=2, space="PSUM") as pt2:
            for n in range(NL):
                qT = qtp.tile([128, T], BF16)
                for th in range(2):
                    sl = slice(th * 512, (th + 1) * 512)
                    pq = pq2.tile([128, 512], dt)
                    for c in range(DC):
                        nc.tensor.matmul(pq, wq_sbs[n][:, c, :], xTs[c][:, sl],
                                         start=(c == 0), stop=(c == DC - 1))
                    tmp = ropet.tile([128, 512], dt)
                    stage = ropet.tile([128, 512], dt, tag="qstage",
                                       name="qstage")
                    _rope(nc, qT[:, sl], pq, cosq[:, sl], sinq[:, sl], tmp,
                          stage)
                if dbg and n == 0:
                    nc.scalar.dma_start(out=q0_dbg, in_=qT)
                    nc.scalar.dma_start(out=kprobe[0], in_=kT)
                for qb in range(4):          # q blocks of 256 rows
                    R = qb * 256
                    d1 = 2 * qb + 1          # last (diagonal) chunk
                    d0 = d1 - 1              # diagonal chunk of sub0
                    nch = d1 + 1
                    # pe0/pe1 share one bank: pe0's kc==0 matmul carries
                    # the bank's single start (zeroing both regions), pe1's
                    # kc==d1 matmul its single stop
                    pe = pe2.tile([128, 264], dt)
                    pe0, pe1 = pe[:, 0:129], pe[:, 132:261]
                    groups = [(k0, k0 + 2) for k0 in range(0, nch, 2)]
                    for (k0, k1) in groups:
                        W = (k1 - k0) * 256
                        ps = pl2.tile([128, 512], dt)
                        # one bank: single start (zeroes it) on the first
                        # chunk, stop on the last; second chunk accumulates
                        # into its zeroed half
                        for kc in range(k0, k1):
                            o = (kc - k0) * 256
                            nc.tensor.matmul(ps[:, o:o + 256],
                                             kT[:, kc * 128:(kc + 1) * 128],
                                             qT[:, R:R + 256],
                                             start=(kc == k0),
                                             stop=(kc == k1 - 1))
                        ex = expp.tile([128, 512], BF16)
                        nc.scalar.activation(
                            out=ex[:, 0:W], in_=ps[:, 0:W],
                            func=mybir.ActivationFunctionType.Exp)
                        if k0 <= d0 < k1:
                            o = (d0 - k0) * 256
                            nc.vector.tensor_mul(ex[:, o:o + 128],
                                                 ex[:, o:o + 128],
                                                 tri[:, 0:128])
                        if k0 <= d1 < k1:
                            o = (d1 - k0) * 256
                            nc.vector.tensor_mul(ex[:, o:o + 256],
                                                 ex[:, o:o + 256],
                                                 tri[:, 128:384])
                        for kc in range(k0, k1):
                            o = (kc - k0) * 256
                            if kc <= d0:
                                nc.tensor.matmul(pe0, ex[:, o:o + 128],
                                                 vext[:, kc, :],
                                                 start=(kc == 0),
                                                 stop=False)
                            nc.tensor.matmul(pe1, ex[:, o + 128:o + 256],
                                             vext[:, kc, :],
                                             start=False, stop=(kc == d1))
                    for s, pes in ((0, pe0), (1, pe1)):
                        ts = 2 * qb + s
                        rc = recp.tile([128, 1], dt)
                        nc.vector.reciprocal(rc, pes[:, 128:129])
                        en = encp.tile([128, 128], BF16)
                        nc.vector.tensor_scalar_mul(en, pes[:, 0:128], rc)
                        ptt = pt2.tile([128, 128], BF16)
                        nc.tensor.transpose(ptt, en, ident)
                        nc.vector.tensor_copy(out=encT[:, n, ts, :], in_=ptt)
                if dbg and n in (0, 3) and qb == 3:
                    nc.scalar.dma_start(out=kprobe[1 if n == 0 else 2],
                                        in_=kT)

        if dbg:
            nc.scalar.dma_start(out=kT_dbg2, in_=kT)
            nc.scalar.dma_start(out=vext_dbg2, in_=vext)
            nc.scalar.dma_start(out=encT_dbg, in_=encT)

        # ---- phase 3: output projection, accumulate over heads per row-block ----
        with tc.tile_pool(name="outp", bufs=2) as outp, \
             tc.tile_pool(name="po3", bufs=2, space="PSUM") as po3:
            for ts in range(TT):
                po = po3.tile([128, 2048], dt)
                for n in range(NL):
                    for c4 in range(4):
                        nc.tensor.matmul(
                            po[:, c4 * 512:(c4 + 1) * 512],
                            encT[:, n, ts, :],
                            wo_sbs[n][:, c4 * 512:(c4 + 1) * 512],
                            start=(n == 0), stop=(n == NL - 1))
                ob = outp.tile([128, 2048], BF16)
                nc.vector.tensor_copy(out=ob[:, 0:1024], in_=po[:, 0:1024])
                nc.scalar.copy(out=ob[:, 1024:2048], in_=po[:, 1024:2048])
                nc.sync.dma_start(out=out_d[ts * 128:(ts + 1) * 128, :],
                                  in_=ob)
    nc.compile()
    return nc


def make_in_maps(x, wq, wkv, wo, segment_pos, attn_mask):
    x = np.asarray(x, dtype=np.float32)
    wq = np.asarray(wq, dtype=np.float32)
    wkv = np.asarray(wkv, dtype=np.float32)
    wo = np.asarray(wo, dtype=np.float32)
    segment_pos = np.asarray(segment_pos)
    attn_mask = np.asarray(attn_mask)

    # rope-pair interleave permutation (see SHUF_MASK): lane j of quadrant qd
    # holds orig dim qd*16+(j%16) for lanes 0-15, 64+qd*16+(j%16) for 16-31.
    lanes = np.arange(HD)
    qd, lane = lanes // 32, lanes % 32
    f = qd * 16 + (lane % 16)
    perm = np.where(lane < 16, f, HHD + f)
    sgn = np.where(lane < 16, np.float32(-1.0), np.float32(1.0))

    def _pch(w):     # [D, H] -> [128, DC, H] with D = (c p)
        return np.ascontiguousarray(
            w.reshape(DC, 128, HD).transpose(1, 0, 2).astype(NP_BF16))

    wk = _pch(wkv[0, 0][:, perm])
    wv = _pch(wkv[1, 0])
    frac = (2.0 / HD) * np.arange(HHD, dtype=np.float32)
    timescale = (np.float32(10000.0) ** frac).astype(np.float32)
    scale = np.float32(HD ** -0.5)

    in_maps = []
    for c in range(8):
        b, g = c // 2, c % 2
        pos = segment_pos[b].astype(np.float32)
        sinus = pos[:, None] / timescale[None, :]          # [T, 64]
        cos = np.cos(sinus).astype(np.float32).T           # [64, T]
        sin = np.sin(sinus).astype(np.float32).T
        cosD = cos[f, :]                                   # [128, T]
        sinS = sgn[:, None] * sin[f, :]
        tri1 = attn_mask[b, :128, :128].T.astype(NP_BF16)  # [k, q] lower-left
        tri = np.zeros((128, 384), dtype=NP_BF16)
        tri[:, 0:128] = tri1
        tri[:, 256:384] = tri1
        xT = np.ascontiguousarray(
            x[b].astype(NP_BF16).T.reshape(DC, 128, T))
        in_maps.append({
            "xT": xT,
            "wq": np.stack([_pch(wq[g * NL + n][:, perm])
                            for n in range(NL)]),
            "wk": wk,
            "wv": wv,
            "wo": np.ascontiguousarray(
                wo[g * NL:(g + 1) * NL].astype(NP_BF16)),
            "cosq": np.ascontiguousarray(cosD * scale),
            "sinq": np.ascontiguousarray(sinS * scale),
            "cosk": np.ascontiguousarray(cosD),
            "sink": np.ascontiguousarray(sinS),
            "tri": tri,
        })
    return in_maps


_NC_CACHE = None


def kernel(**inputs):
    global _NC_CACHE
    if _NC_CACHE is None:
        _NC_CACHE = build_nc()
    nc = _NC_CACHE
    in_maps = make_in_maps(
        inputs["x"], inputs["wq"], inputs["wkv"], inputs["wo"],
        inputs["segment_pos"], inputs["attn_mask"])
    res = run_bass_kernel_spmd(nc, in_maps, core_ids=list(range(8)))
    out = np.empty((B, T, D), dtype=np.float32)
    for b in range(B):
        out[b] = (res.results[2 * b]["out"].astype(np.float32)
                  + res.results[2 * b + 1]["out"].astype(np.float32))
    return out
